# revision 1
# baseline (speedup 1.0000x reference)
"""Trainium2 Bass kernel for LayerNorm + multi-head attention (B=4, S=2048,
D=1024, H=16) with the sigmoid(s-mu)*exp(s) row-normalized attention weights.

Sharding: 8 cores = 4 batches x 2 head-groups (8 heads each). Each core
computes LN + its head-group's q/k/v projections + attention + a partial
output projection; the host sums the two partials per batch and adds bo.

Key device-side identities:
  p_i = sigmoid(s_i - mu) * exp(s_i) / sum_j(...)
      = g(z_i) / sum_j g(z_j),   z = s - mu,  g(z) = e^z * sigmoid(z)
  (the e^mu factor cancels in the normalization), and
  1 / g(z) = y * (1 + y)  with  y = e^{-z}
so the per-element work is ONE ScalarE exp plus ONE fused custom-DVE op
computing 1/(x + x^2) (bitwise-not reciprocal seed + one Newton step).
"""

import sys

if "/opt/trn_rl_repo" not in sys.path:
    sys.path.insert(0, "/opt/trn_rl_repo")

import numpy as np
import ml_dtypes as _ml

D_MODEL = 1024
N_HEADS = 16
HEAD_DIM = 64
SEQ = 2048
BATCH = 4
N_CORES = 8
EPS = 1e-6
SCALE = float(HEAD_DIM) ** 0.5  # 8.0

# Per-core partitioning
H_LOC = 8          # heads per core
W_LOC = H_LOC * HEAD_DIM  # 512 local projection width
N_DT = D_MODEL // 128     # 8 d-tiles
N_TT = SEQ // 128         # 16 token tiles
N_TC = 4                  # token chunks of 512
VSTR = 66                 # per-head stride in v_aug ([64 v | 1 ones | 1 pad])

# Custom-DVE fused reciprocal-of-act2 constants (Chebyshev-ish, tuned for
# the w*bitcast(~w) in [-4.5,-4] seed interval; ~0.4% max rel err).
RA_C0 = -0.234
RA_C1 = 2.0
_DEBUG = False


def _get_recip_act2_op():
    """Register (once) and return the custom DVE op: out = 1/(x + x^2)."""
    import concourse.dve_ops as dve_ops

    if hasattr(dve_ops, "RECIP_ACT2_ANT"):
        return dve_ops.RECIP_ACT2_ANT

    from concourse.dve_spec import Spec, Src0, C0, C1, Bin, AluOp, sq, lower, _has_src1
    from concourse.dve_uop import DveOpSpec

    _w = sq(Src0) + Src0
    _nw = Bin(AluOp.BITWISE_NOT, _w, _w)
    _y0 = _nw * C0
    _body = _y0 * (C1 - _w * _y0)

    def _ref(in0, in1, s0, s1, imm2):
        x = np.asarray(in0).astype(np.float32)
        w = (x + x * x).astype(np.float32)
        nw = (~w.view(np.int32)).view(np.float32)
        if isinstance(s0, np.ndarray):
            s0 = s0.astype(np.float32)
        if isinstance(s1, np.ndarray):
            s1 = s1.astype(np.float32)
        y0 = (nw * np.float32(s0) if not isinstance(s0, np.ndarray) else nw * s0).astype(np.float32)
        c1 = np.float32(s1) if not isinstance(s1, np.ndarray) else s1
        return (y0 * (c1 - w * y0)).astype(np.float32)

    spec = Spec(body=_body, reference=_ref)
    name = "RECIP_ACT2_ANT"
    row = max(dve_ops._SUB_OPCODE_FOR_NAME.values()) + 1
    assert row < 0x20
    dve_ops._SUB_OPCODE_FOR_NAME[name] = row
    shas = {}
    for ver in ("v3", "v4"):
        compiled = DveOpSpec(
            name=name, opcode=row, uops=lower(spec, ver=ver), rd1_en=_has_src1(spec)
        )
        shas[ver] = compiled.sha(ver)
    op = dve_ops.DveOp(name, spec, subdim=False, uops_sha=shas)
    dve_ops.OPS.append(op)
    dve_ops.CUSTOM_DVE_SPECS[name] = spec
    dve_ops.RECIP_ACT2_ANT = op
    return op


def _broadcast_ap(ap, parts):
    """Partition-broadcast a 1-D AP of shape [N] to [parts, N] (step-0)."""
    import concourse.bass as bass

    steps = [list(p) for p in ap.ap]
    return bass.AP(tensor=ap.tensor, offset=ap.offset, ap=[[0, parts]] + steps)


def _build_program(mu_val: float):
    import concourse.bass as bass
    import concourse.mybir as mybir
    import concourse.tile as tile
    from concourse import bacc
    from concourse.masks import make_identity
    from concourse.dve_ops import RECIPROCAL_APPROX_FAST, RECIP_APPROX_FAST_CONSTS

    recip_act2 = _get_recip_act2_op()

    f32 = mybir.dt.float32
    f32r = mybir.dt.float32r
    bf16 = mybir.dt.bfloat16
    AF = mybir.ActivationFunctionType
    ALU = mybir.AluOpType

    nc = bacc.Bacc("TRN2", target_bir_lowering=False, debug=False,
                   num_devices=N_CORES)

    x_d = nc.dram_tensor("x", [SEQ, D_MODEL], f32, kind="ExternalInput").ap()
    wq_d = nc.dram_tensor("wqT", [D_MODEL, W_LOC], f32, kind="ExternalInput").ap()
    wk_d = nc.dram_tensor("wkT", [D_MODEL, W_LOC], f32, kind="ExternalInput").ap()
    wv_d = nc.dram_tensor("wvT", [D_MODEL, W_LOC], f32, kind="ExternalInput").ap()
    wo_d = nc.dram_tensor("woT", [W_LOC, D_MODEL], bf16, kind="ExternalInput").ap()
    bq_d = nc.dram_tensor("bq", [W_LOC], f32, kind="ExternalInput").ap()
    bk_d = nc.dram_tensor("bk", [W_LOC], f32, kind="ExternalInput").ap()
    bv_d = nc.dram_tensor("bv", [W_LOC], f32, kind="ExternalInput").ap()
    out_d = nc.dram_tensor("out", [SEQ, D_MODEL], f32, kind="ExternalOutput").ap()
    dbg = {}
    if _DEBUG:
        dbg["qT"] = nc.dram_tensor("dbg_qT", [128, 4, SEQ], f32, kind="ExternalOutput").ap()
        dbg["kT"] = nc.dram_tensor("dbg_kT", [128, 4, SEQ], f32, kind="ExternalOutput").ap()
        dbg["v"] = nc.dram_tensor("dbg_v", [128, N_TT, H_LOC * VSTR], bf16, kind="ExternalOutput").ap()
        dbg["attn"] = nc.dram_tensor("dbg_attn", [128, 4, SEQ], bf16, kind="ExternalOutput").ap()

    rc = RECIP_APPROX_FAST_CONSTS

    with tile.TileContext(nc) as tc:
        with (
            tc.tile_pool(name="consts", bufs=1) as consts,
            tc.tile_pool(name="qkv", bufs=1) as qkv_pool,
        ):
            ident = consts.tile([128, 128], f32)
            make_identity(nc, ident)
            ones_f32 = consts.tile([128, 64], f32)
            nc.vector.memset(ones_f32, 1.0)
            ones_sb = consts.tile([128, 64], f32r)
            nc.vector.tensor_copy(out=ones_sb, in_=ones_f32)
            eps_sb = consts.tile([128, 1], f32)
            nc.vector.memset(eps_sb, EPS)
            bq_sb = consts.tile([128, 4], f32)
            nc.sync.dma_start(out=bq_sb, in_=bq_d.rearrange("(a p) -> p a", p=128))
            bk_sb = consts.tile([128, 4], f32)
            nc.sync.dma_start(out=bk_sb, in_=bk_d.rearrange("(a p) -> p a", p=128))
            bv_bc = consts.tile([128, W_LOC], f32)
            nc.sync.dma_start(out=bv_bc, in_=_broadcast_ap(bv_d, 128))

            # persistent activations
            qT = qkv_pool.tile([128, 4, SEQ], f32r)  # [j, t] (4 j-tiles)
            kT = qkv_pool.tile([128, 4, SEQ], f32r)
            v_aug = qkv_pool.tile([128, N_TT, H_LOC * VSTR], bf16)
            # ones (and pad) columns for the AV numerator+denominator trick
            nc.vector.memset(
                v_aug.rearrange("p t (h c) -> p t h c", c=VSTR)[:, :, :, 64:66], 1.0
            )

            # ---------------- Phase A: LN + transpose + projections --------
            with (
                tc.tile_pool(name="wqkv", bufs=1) as wp,
                tc.tile_pool(name="wtmp", bufs=2) as wtp,
                tc.tile_pool(name="ph_x", bufs=2) as xp,
                tc.tile_pool(name="ph_s", bufs=3) as sp,
                tc.tile_pool(name="xt", bufs=1) as xtp,
                tc.tile_pool(name="psA", bufs=2, space="PSUM") as psA,
                tc.tile_pool(name="psT", bufs=2, space="PSUM") as psT,
            ):
                w_sbs = {}
                for wname, w_d in (("q", wq_d), ("k", wk_d), ("v", wv_d)):
                    w_sbs[wname] = wp.tile([128, N_DT, W_LOC], f32r,
                                           name=f"w{wname}", tag=f"w{wname}")
                    for half in range(2):
                        hs = slice(half * 4, half * 4 + 4)
                        wtmp = wtp.tile([128, 4, W_LOC], f32, tag="wtmp")
                        nc.sync.dma_start(
                            out=wtmp,
                            in_=w_d.rearrange("(a p) j -> p a j", p=128)[:, hs, :])
                        nc.vector.tensor_copy(out=w_sbs[wname][:, hs, :], in_=wtmp)
                wq_sb, wk_sb, wv_sb = w_sbs["q"], w_sbs["k"], w_sbs["v"]

                for tc_i in range(N_TC):  # chunks of 512 tokens
                    xT = xtp.tile([128, N_DT, 512], f32r, tag="xT")
                    for i in range(4):  # 128-token subtiles
                        tt = tc_i * 4 + i
                        x_t = xp.tile([128, D_MODEL], f32, tag="x")
                        nc.sync.dma_start(out=x_t, in_=x_d[tt * 128:(tt + 1) * 128, :])
                        stats = sp.tile([128, 2, 6], f32, tag="bn")
                        nc.vector.bn_stats(out=stats[:, 0, :], in_=x_t[:, 0:512])
                        nc.vector.bn_stats(out=stats[:, 1, :], in_=x_t[:, 512:1024])
                        mv = sp.tile([128, 2], f32, tag="mv")
                        nc.vector.bn_aggr(out=mv, in_=stats)
                        rstd = sp.tile([128, 1], f32, tag="rstd")
                        nc.scalar.activation(out=rstd, in_=mv[:, 1:2], func=AF.Sqrt,
                                             bias=eps_sb, scale=1.0)
                        nc.vector.reciprocal(out=rstd, in_=rstd)
                        nmr = sp.tile([128, 1], f32, tag="nmr")
                        nc.vector.tensor_scalar(out=nmr, in0=mv[:, 0:1], scalar1=rstd,
                                                scalar2=-1.0, op0=ALU.mult, op1=ALU.mult)
                        xs_t = xp.tile([128, D_MODEL], f32, tag="xs")
                        nc.vector.tensor_scalar(out=xs_t, in0=x_t, scalar1=rstd,
                                                scalar2=nmr, op0=ALU.mult, op1=ALU.add)
                        for d in range(N_DT):
                            ps_tr = psT.tile([128, 128], f32, tag="tr")
                            nc.tensor.transpose(ps_tr, xs_t[:, d * 128:(d + 1) * 128], ident)
                            if d % 2 == 0:
                                nc.scalar.copy(out=xT[:, d, i * 128:(i + 1) * 128], in_=ps_tr)
                            else:
                                nc.vector.tensor_copy(out=xT[:, d, i * 128:(i + 1) * 128], in_=ps_tr)

                    # q/k projections for this chunk: out[jt, t] in PSUM
                    for which, w_sb, b_sb, dstT in (("q", wq_sb, bq_sb, qT), ("k", wk_sb, bk_sb, kT)):
                        for jt in range(4):
                            ps = psA.tile([128, 512], f32, tag="pj")
                            for d in range(N_DT):
                                nc.tensor.matmul(
                                    ps,
                                    w_sb[:, d, jt * 128:(jt + 1) * 128],
                                    xT[:, d, :],
                                    start=(d == 0), stop=(d == N_DT - 1),
                                )
                            nc.scalar.activation(
                                out=dstT[:, jt, tc_i * 512:(tc_i + 1) * 512], in_=ps,
                                func=AF.Identity, bias=b_sb[:, jt:jt + 1], scale=1.0)

                    # v projection: out[t, j] + bias, written bf16 into v_aug
                    for i in range(4):
                        tt = tc_i * 4 + i
                        ps = psA.tile([128, 512], f32, tag="pv")
                        for d in range(N_DT):
                            nc.tensor.matmul(
                                ps,
                                xT[:, d, i * 128:(i + 1) * 128],
                                wv_sb[:, d, :],
                                start=(d == 0), stop=(d == N_DT - 1),
                            )
                        v_view = v_aug.rearrange("p t (h c) -> p t h c", c=VSTR)[:, tt, :, 0:64]
                        nc.vector.tensor_tensor(
                            out=v_view, in0=ps.rearrange("p (h c) -> p h c", c=64),
                            in1=bv_bc.rearrange("p (h c) -> p h c", c=64), op=ALU.add)

            # ---------------- Phase B: attention ---------------------------
            with tc.tile_pool(name="attn", bufs=1) as attn_pool:
                attn_sb = attn_pool.tile([128, 4, SEQ], bf16)  # [d_local, t]

                _phase_b(tc, nc, tile, mybir, qT, kT, v_aug, attn_sb, ones_sb,
                         mu_val, recip_act2, RECIPROCAL_APPROX_FAST, rc)
                _phase_c(tc, nc, mybir, attn_sb, wo_d, out_d)
                if _DEBUG:
                    nc.sync.dma_start(out=dbg["qT"], in_=qT.bitcast(f32))
                    nc.sync.dma_start(out=dbg["kT"], in_=kT.bitcast(f32))
                    nc.sync.dma_start(out=dbg["v"], in_=v_aug)
                    nc.sync.dma_start(out=dbg["attn"], in_=attn_sb)

    nc.compile()
    return nc


def _phase_b(tc, nc, tile, mybir, qT, kT, v_aug, attn_sb, ones_sb, mu_val,
             recip_act2, RECIPROCAL_APPROX_FAST, rc):
    f32 = mybir.dt.float32
    f32r = mybir.dt.float32r
    bf16 = mybir.dt.bfloat16
    AF = mybir.ActivationFunctionType
    ALU = mybir.AluOpType
    N_TT = SEQ // 128
    if True:
            with (
                tc.tile_pool(name="gb", bufs=1) as gp,
                tc.tile_pool(name="yb", bufs=4) as yp,
                tc.tile_pool(name="nrm", bufs=2) as nrm,
                tc.tile_pool(name="psS", bufs=2, space="PSUM") as psS,
                tc.tile_pool(name="psV", bufs=1, space="PSUM") as psV,
                tc.tile_pool(name="psB", bufs=1, space="PSUM") as psB,
            ):
                for hp in range(4):  # head pairs; even@rows0-63, odd@rows64-127
                    for t1b in range(4):  # 512-wide t1 blocks
                        t1s = slice(t1b * 512, (t1b + 1) * 512)
                        g_e = gp.tile([128, N_TT, 512], bf16, tag="g_e")
                        g_o = gp.tile([128, N_TT, 512], bf16, tag="g_o")
                        av_e = psV.tile([65, 512], f32, tag="av_e")
                        av_o = psV.tile([65, 512], f32, tag="av_o")
                        for t2t in range(N_TT):
                            for par, g_t, av in ((0, g_e, av_e), (1, g_o, av_o)):
                                rows = slice(par * 64, par * 64 + 64)
                                ps = psS.tile([128, 512], f32, tag=f"sc{par}")
                                nc.tensor.matmul(
                                    ps,
                                    kT[rows, hp, t2t * 128:(t2t + 1) * 128],
                                    qT[rows, hp, t1s],
                                    start=True, stop=True,
                                )
                                y_t = yp.tile([128, 512], f32, tag=f"y{par}")
                                nc.scalar.activation(out=y_t, in_=ps, func=AF.Exp,
                                                     scale=-1.0 / SCALE, bias=mu_val)
                                nc.vector._custom_dve(
                                    recip_act2, out=g_t[:, t2t, :], in0=y_t,
                                    s0=RA_C0, s1=RA_C1)
                                h = hp * 2 + par
                                nc.tensor.matmul(
                                    av,
                                    v_aug[:, t2t, h * VSTR:h * VSTR + 65],
                                    g_t[:, t2t, :],
                                    start=(t2t == 0), stop=(t2t == N_TT - 1),
                                )
                        # normalize: rows 0..63 = sum(g*v); row 64 = sum(g)
                        for par, av in ((0, av_e), (1, av_o)):
                            # custom-DVE ops only work at base partition 0, so
                            # DMA the denominator row down to partition 0 first
                            den64 = nrm.tile([65, 512], f32, tag="den64")
                            nc.scalar.copy(out=den64[64:65, :], in_=av[64:65, :])
                            den_sb = nrm.tile([1, 512], f32, tag="den")
                            nc.sync.dma_start(out=den_sb, in_=den64[64:65, :])
                            r_t = nrm.tile([1, 512], f32, tag="r")
                            nc.vector._custom_dve(
                                RECIPROCAL_APPROX_FAST, out=r_t,
                                in0=den_sb, s0=rc["s0"], s1=rc["s1"],
                                imm2=rc["imm2"])
                            r_r = nrm.tile([1, 512], f32r, tag="rr")
                            nc.vector.tensor_copy(out=r_r, in_=r_t)
                            bc_ps = psB.tile([64, 512], f32, tag="bc")
                            nc.tensor.matmul(
                                bc_ps,
                                ones_sb[0:1, :],
                                r_r,
                                start=True, stop=True,
                            )
                            bc_sb = nrm.tile([64, 512], f32, tag="bc_sb")
                            nc.scalar.copy(out=bc_sb, in_=bc_ps)
                            if par == 0:
                                nc.vector.tensor_tensor(
                                    out=attn_sb[0:64, hp, t1s], in0=av[0:64, :],
                                    in1=bc_sb, op=ALU.mult)
                            else:
                                tmp = nrm.tile([64, 512], bf16, tag="tmp")
                                nc.vector.tensor_tensor(
                                    out=tmp, in0=av[0:64, :], in1=bc_sb, op=ALU.mult)
                                nc.sync.dma_start(out=attn_sb[64:128, hp, t1s], in_=tmp)


def _phase_c(tc, nc, mybir, attn_sb, wo_d, out_d):
    f32 = mybir.dt.float32
    bf16 = mybir.dt.bfloat16
    N_TT = SEQ // 128
    with (
        tc.tile_pool(name="wo", bufs=1) as wop,
        tc.tile_pool(name="ob", bufs=3) as op_,
        tc.tile_pool(name="psO", bufs=3, space="PSUM") as psO,
    ):
        wo_sb = wop.tile([128, 4, D_MODEL], bf16)
        nc.sync.dma_start(out=wo_sb, in_=wo_d.rearrange("(a p) e -> p a e", p=128))
        for et in range(2):
            es = slice(et * 512, (et + 1) * 512)
            for tt2 in range(N_TT):
                ps = psO.tile([128, 512], f32, tag="po")
                for dt in range(4):
                    nc.tensor.matmul(
                        ps,
                        attn_sb[:, dt, tt2 * 128:(tt2 + 1) * 128],
                        wo_sb[:, dt, es],
                        start=(dt == 0), stop=(dt == 3),
                    )
                o_t = op_.tile([128, 512], f32, tag="o")
                nc.scalar.copy(out=o_t, in_=ps)
                nc.sync.dma_start(
                    out=out_d[tt2 * 128:(tt2 + 1) * 128, es], in_=o_t)


_PROGRAM_CACHE = {}


def _get_program(mu_val: float):
    key = round(float(mu_val), 9)
    if key not in _PROGRAM_CACHE:
        _PROGRAM_CACHE[key] = _build_program(float(mu_val))
    return _PROGRAM_CACHE[key]


def make_core_inputs(sequence, ln_gamma, ln_beta, Wq, bq, Wk, bk, Wv, bv, Wo, bo, mu):
    """Host-side shard prep: per-core input dicts (gamma/beta folded into W/b)."""
    f = np.float32
    seq = np.asarray(sequence, f)
    g = np.asarray(ln_gamma, f)
    be = np.asarray(ln_beta, f)
    in_maps = []
    for c in range(N_CORES):
        b, grp = c // 2, c % 2
        blk = slice(W_LOC * grp, W_LOC * (grp + 1))
        Wqb = np.asarray(Wq, f)[blk]
        Wkb = np.asarray(Wk, f)[blk]
        Wvb = np.asarray(Wv, f)[blk]
        m = {
            "x": np.ascontiguousarray(seq[b]),
            "wqT": np.ascontiguousarray((Wqb * g[None, :]).T),
            "wkT": np.ascontiguousarray((Wkb * g[None, :]).T),
            "wvT": np.ascontiguousarray((Wvb * g[None, :]).T),
            "woT": np.ascontiguousarray(np.asarray(Wo, f)[:, blk].T).astype(_ml.bfloat16),
            "bq": np.ascontiguousarray(np.asarray(bq, f)[blk] + Wqb @ be),
            "bk": np.ascontiguousarray(np.asarray(bk, f)[blk] + Wkb @ be),
            "bv": np.ascontiguousarray(np.asarray(bv, f)[blk] + Wvb @ be),
        }
        in_maps.append(m)
    return in_maps


def combine_outputs(results, bo):
    out = np.zeros((BATCH, SEQ, D_MODEL), np.float32)
    for c in range(N_CORES):
        out[c // 2] += results[c]["out"]
    out += np.asarray(bo, np.float32)[None, None, :]
    return out


def kernel(sequence, ln_gamma, ln_beta, Wq, bq, Wk, bk, Wv, bv, Wo, bo, mu,
           _trace=False):
    from concourse.bass_utils import run_bass_kernel_spmd

    mu_val = float(np.asarray(mu).reshape(-1)[0])
    nc = _get_program(mu_val)
    in_maps = make_core_inputs(sequence, ln_gamma, ln_beta, Wq, bq, Wk, bk,
                               Wv, bv, Wo, bo, mu)
    res = run_bass_kernel_spmd(nc, in_maps, list(range(N_CORES)), trace=_trace)
    out = combine_outputs(res.results, bo)
    if _trace:
        kernel.last_results = res
    return out



# revision 8
# speedup vs baseline: 1.0291x; 1.0291x over previous
"""Trainium2 Bass kernel for LayerNorm + multi-head attention (B=4, S=2048,
D=1024, H=16) with sigmoid(s-mu)*exp(s) row-normalized attention weights.

Sharding: 8 cores = 4 batches x 2 head-groups (8 heads each). Each core
computes LN + its head-group's q/k/v projections + attention + a partial
output projection; the host sums the two partials per batch and adds bo.

Device-side identities:
  p_i = g(z_i) / sum_j g(z_j),  z = s - mu,  g(z) = e^z * sigmoid(z)
  1 / g(z) = y * (1 + y)  with  y = e^{-z}
so per score element: ONE ScalarE exp + ONE fused custom-DVE op computing
1/(x + x^2) (bitwise-not reciprocal seed + one Newton step).

v2 layout/scheduling:
  - all matmul operands bf16 (host-prepped weights; LN output cast bf16)
  - score matmuls for the even/odd head of a pair are K=64 row-tiled
    (tile_position (0,0)/(64,0)) into one [128,2,512] 2-bank PSUM tile;
    ONE exp + ONE recip-act2 call covers both parities (1024 cols)
  - AV matmuls (with the ones-column denominator trick) are emitted LAG
    steps behind the score matmuls so the in-order PE queue never stalls
    waiting on the DVE's g tiles
  - normalization: denominator rows DMA'd to partition 0/1, one fast
    reciprocal (DVE), broadcast + av rows moved to SBUF by DMA, final
    multiply on the (otherwise idle) Pool/GpSimd engine
"""

import sys

if "/opt/trn_rl_repo" not in sys.path:
    sys.path.insert(0, "/opt/trn_rl_repo")

import numpy as np
import ml_dtypes as _ml

D_MODEL = 1024
N_HEADS = 16
HEAD_DIM = 64
SEQ = 2048
BATCH = 4
N_CORES = 8
EPS = 1e-6
SCALE = float(HEAD_DIM) ** 0.5  # 8.0

# Per-core partitioning
H_LOC = 8          # heads per core
W_LOC = H_LOC * HEAD_DIM  # 512 local projection width
N_DT = D_MODEL // 128     # 8 d-tiles
N_TT = SEQ // 128         # 16 token tiles (t2 direction)
N_TB = 4                  # t1 blocks of 512
VSTR = 66                 # per-head stride in v_aug ([64 v | 1 ones | 1 pad])
LAG = 2                   # AV matmuls trail score matmuls by LAG k-tiles

# Custom-DVE fused reciprocal-of-act2 constants (tuned for the
# w*bitcast(~w) seed interval; ~0.4% max rel err).
RA_C0 = -0.234
RA_C1 = 2.0


def _get_recip_act2_op():
    """Register (once) and return the custom DVE op: out = 1/(x + x^2)."""
    import concourse.dve_ops as dve_ops

    if hasattr(dve_ops, "RECIP_ACT2_ANT"):
        return dve_ops.RECIP_ACT2_ANT

    from concourse.dve_spec import Spec, Src0, C0, C1, Bin, AluOp, sq, lower, _has_src1
    from concourse.dve_uop import DveOpSpec

    _w = sq(Src0) + Src0
    _nw = Bin(AluOp.BITWISE_NOT, _w, _w)
    _y0 = _nw * C0
    _body = _y0 * (C1 - _w * _y0)

    def _ref(in0, in1, s0, s1, imm2):
        x = np.asarray(in0).astype(np.float32)
        w = (x + x * x).astype(np.float32)
        nw = (~w.view(np.int32)).view(np.float32)
        if isinstance(s0, np.ndarray):
            s0 = s0.astype(np.float32)
        if isinstance(s1, np.ndarray):
            s1 = s1.astype(np.float32)
        y0 = (nw * np.float32(s0) if not isinstance(s0, np.ndarray) else nw * s0).astype(np.float32)
        c1 = np.float32(s1) if not isinstance(s1, np.ndarray) else s1
        return (y0 * (c1 - w * y0)).astype(np.float32)

    spec = Spec(body=_body, reference=_ref)
    name = "RECIP_ACT2_ANT"
    row = max(dve_ops._SUB_OPCODE_FOR_NAME.values()) + 1
    assert row < 0x20
    dve_ops._SUB_OPCODE_FOR_NAME[name] = row
    shas = {}
    for ver in ("v3", "v4"):
        compiled = DveOpSpec(
            name=name, opcode=row, uops=lower(spec, ver=ver), rd1_en=_has_src1(spec)
        )
        shas[ver] = compiled.sha(ver)
    op = dve_ops.DveOp(name, spec, subdim=False, uops_sha=shas)
    dve_ops.OPS.append(op)
    dve_ops.CUSTOM_DVE_SPECS[name] = spec
    dve_ops.RECIP_ACT2_ANT = op
    return op


def _broadcast_ap(ap, parts):
    """Partition-broadcast a 1-D AP of shape [N] to [parts, N] (step-0)."""
    import concourse.bass as bass

    steps = [list(p) for p in ap.ap]
    return bass.AP(tensor=ap.tensor, offset=ap.offset, ap=[[0, parts]] + steps)


def _broadcast_row(ap2d, parts):
    """Partition-broadcast a [1, N] AP to [parts, N] (step-0 partition dim)."""
    import concourse.bass as bass

    steps = [list(p) for p in ap2d.ap[1:]]
    return bass.AP(tensor=ap2d.tensor, offset=ap2d.offset, ap=[[0, parts]] + steps)


def _build_program(mu_val: float):
    import concourse.bass as bass
    import concourse.mybir as mybir
    import concourse.tile as tile
    from concourse import bacc
    from concourse.masks import make_identity
    from concourse.dve_ops import RECIPROCAL_APPROX_FAST, RECIP_APPROX_FAST_CONSTS

    recip_act2 = _get_recip_act2_op()

    f32 = mybir.dt.float32
    bf16 = mybir.dt.bfloat16
    AF = mybir.ActivationFunctionType
    ALU = mybir.AluOpType

    nc = bacc.Bacc("TRN2", target_bir_lowering=False, debug=False,
                   num_devices=N_CORES)

    x_d = nc.dram_tensor("x", [SEQ, D_MODEL], f32, kind="ExternalInput").ap()
    wq_d = nc.dram_tensor("wqT", [D_MODEL, W_LOC], bf16, kind="ExternalInput").ap()
    wk_d = nc.dram_tensor("wkT", [D_MODEL, W_LOC], bf16, kind="ExternalInput").ap()
    wv_d = nc.dram_tensor("wvT", [D_MODEL, W_LOC], bf16, kind="ExternalInput").ap()
    wo_d = nc.dram_tensor("woT", [W_LOC, D_MODEL], bf16, kind="ExternalInput").ap()
    bq_d = nc.dram_tensor("bq", [W_LOC], f32, kind="ExternalInput").ap()
    bk_d = nc.dram_tensor("bk", [W_LOC], f32, kind="ExternalInput").ap()
    bv_d = nc.dram_tensor("bv", [W_LOC], f32, kind="ExternalInput").ap()
    out_d = nc.dram_tensor("out", [SEQ, D_MODEL], f32, kind="ExternalOutput").ap()
    # DRAM bounce buffer for the per-block reciprocal rows (DMA partition-
    # broadcast requires a DRAM source); one row per (hp, t1b) block.
    rsc_d = nc.dram_tensor("rscratch", [16, 2, 512], bf16, kind="Internal").ap()

    rc = RECIP_APPROX_FAST_CONSTS

    with tile.TileContext(nc) as tc:
        with (
            tc.tile_pool(name="consts", bufs=1) as consts,
            tc.tile_pool(name="qkv", bufs=1) as qkv_pool,
        ):
            ident = consts.tile([128, 128], bf16)
            make_identity(nc, ident)
            eps_sb = consts.tile([128, 1], f32)
            nc.vector.memset(eps_sb, EPS)
            bq_sb = consts.tile([128, 4], f32)
            nc.sync.dma_start(out=bq_sb, in_=bq_d.rearrange("(a p) -> p a", p=128))
            bk_sb = consts.tile([128, 4], f32)
            nc.sync.dma_start(out=bk_sb, in_=bk_d.rearrange("(a p) -> p a", p=128))
            bv_bc = consts.tile([128, W_LOC], f32)
            nc.sync.dma_start(out=bv_bc, in_=_broadcast_ap(bv_d, 128))

            # persistent activations (all bf16)
            qT = qkv_pool.tile([128, 4, SEQ], bf16)   # [pair-dim, hp, t]
            kT = qkv_pool.tile([128, 4, SEQ], bf16)
            v_aug = qkv_pool.tile([128, N_TT, H_LOC * VSTR], bf16)
            attn_sb = qkv_pool.tile([128, 4, SEQ], bf16)
            # ones (and pad) columns for the AV numerator+denominator trick
            nc.vector.memset(
                v_aug.rearrange("p t (h c) -> p t h c", c=VSTR)[:, :, :, 64:66], 1.0
            )

            # weights in SBUF, straight bf16 DMA
            wq_sb = qkv_pool.tile([128, N_DT, W_LOC], bf16)
            nc.sync.dma_start(out=wq_sb, in_=wq_d.rearrange("(a p) j -> p a j", p=128))
            wk_sb = qkv_pool.tile([128, N_DT, W_LOC], bf16)
            nc.sync.dma_start(out=wk_sb, in_=wk_d.rearrange("(a p) j -> p a j", p=128))
            wv_sb = qkv_pool.tile([128, N_DT, W_LOC], bf16)
            nc.sync.dma_start(out=wv_sb, in_=wv_d.rearrange("(a p) j -> p a j", p=128))
            xT = qkv_pool.tile([128, N_DT, SEQ], bf16)  # [d, tokens] transposed LN(x)

            # ------------- Phase A: LN + transpose -------------------------
            with (
                tc.tile_pool(name="ph_x", bufs=2) as xp,
                tc.tile_pool(name="ph_s", bufs=3) as sp,
                tc.tile_pool(name="psT", bufs=3, space="PSUM") as psT,
            ):
                for tt in range(N_TT):
                    x_t = xp.tile([128, D_MODEL], f32, tag="x")
                    nc.sync.dma_start(out=x_t, in_=x_d[tt * 128:(tt + 1) * 128, :])
                    stats = sp.tile([128, 2, 6], f32, tag="bn")
                    nc.vector.bn_stats(out=stats[:, 0, :], in_=x_t[:, 0:512])
                    nc.vector.bn_stats(out=stats[:, 1, :], in_=x_t[:, 512:1024])
                    mv = sp.tile([128, 2], f32, tag="mv")
                    nc.vector.bn_aggr(out=mv, in_=stats)
                    rstd = sp.tile([128, 1], f32, tag="rstd")
                    nc.scalar.activation(out=rstd, in_=mv[:, 1:2], func=AF.Sqrt,
                                         bias=eps_sb, scale=1.0)
                    nc.vector.reciprocal(out=rstd, in_=rstd)
                    nmr = sp.tile([128, 1], f32, tag="nmr")
                    nc.vector.tensor_scalar(out=nmr, in0=mv[:, 0:1], scalar1=rstd,
                                            scalar2=-1.0, op0=ALU.mult, op1=ALU.mult)
                    xs_t = xp.tile([128, D_MODEL], bf16, tag="xs")
                    nc.vector.tensor_scalar(out=xs_t, in0=x_t, scalar1=rstd,
                                            scalar2=nmr, op0=ALU.mult, op1=ALU.add)
                    for d in range(N_DT):
                        ps_tr = psT.tile([128, 128], bf16, tag="tr")
                        nc.tensor.transpose(ps_tr, xs_t[:, d * 128:(d + 1) * 128], ident)
                        if d % 2 == 0:
                            nc.scalar.copy(
                                out=xT[:, d, tt * 128:(tt + 1) * 128], in_=ps_tr)
                        else:
                            nc.vector.tensor_copy(
                                out=xT[:, d, tt * 128:(tt + 1) * 128], in_=ps_tr)

            # ------------- Phase A2: projections ---------------------------
            with (
                tc.tile_pool(name="psA", bufs=2, space="PSUM") as psA,
            ):
                # v projection: out[t, j] + bias -> v_aug bf16
                for tt in range(N_TT):
                    ps = psA.tile([128, 512], f32, tag="pj")
                    for d in range(N_DT):
                        nc.tensor.matmul(
                            ps,
                            xT[:, d, tt * 128:(tt + 1) * 128],
                            wv_sb[:, d, :],
                            start=(d == 0), stop=(d == N_DT - 1),
                        )
                    v_view = v_aug.rearrange("p t (h c) -> p t h c", c=VSTR)[:, tt, :, 0:64]
                    nc.vector.tensor_tensor(
                        out=v_view, in0=ps.rearrange("p (h c) -> p h c", c=64),
                        in1=bv_bc.rearrange("p (h c) -> p h c", c=64), op=ALU.add)

                # q/k projections, hp-major: out[jt, t] in PSUM -> bf16 SBUF
                for hp in range(4):
                    for w_sb, b_sb, dstT in ((wq_sb, bq_sb, qT), (wk_sb, bk_sb, kT)):
                        for cb in range(4):  # 512-token chunks
                            ps = psA.tile([128, 512], f32, tag="pj")
                            for d in range(N_DT):
                                nc.tensor.matmul(
                                    ps,
                                    w_sb[:, d, hp * 128:(hp + 1) * 128],
                                    xT[:, d, cb * 512:(cb + 1) * 512],
                                    start=(d == 0), stop=(d == N_DT - 1),
                                )
                            nc.scalar.activation(
                                out=dstT[:, hp, cb * 512:(cb + 1) * 512], in_=ps,
                                func=AF.Identity, bias=b_sb[:, hp:hp + 1], scale=1.0)

            # ------------- Phase B: attention ------------------------------
            _phase_b(tc, nc, tile, mybir, qT, kT, v_aug, attn_sb, rsc_d,
                     mu_val, recip_act2, RECIPROCAL_APPROX_FAST, rc)

            # ------------- Phase C: output projection ----------------------
            _phase_c(tc, nc, mybir, attn_sb, wo_d, out_d)

    nc.compile()
    return nc


def _phase_b(tc, nc, tile, mybir, qT, kT, v_aug, attn_sb, rsc_d, mu_val,
             recip_act2, RECIPROCAL_APPROX_FAST, rc):
    f32 = mybir.dt.float32
    bf16 = mybir.dt.bfloat16
    AF = mybir.ActivationFunctionType
    ALU = mybir.AluOpType

    blocks = [(hp, t1b) for hp in range(4) for t1b in range(4)]
    n_steps = len(blocks) * N_TT

    with (
        tc.tile_pool(name="yb", bufs=2) as yp,
        tc.tile_pool(name="gb", bufs=LAG + 2) as gp,
        tc.tile_pool(name="nrm", bufs=2) as nrm,
        tc.tile_pool(name="psS", bufs=2, space="PSUM") as psS,
        tc.tile_pool(name="psV", bufs=2, space="PSUM") as psV,
    ):
        g_tiles = {}   # step -> g tile (consumed by the trailing AV pair)
        av_tiles = {}  # block idx -> av [128, 2, 512] (parity on middle axis)

        def emit_lead(step):
            bi, t2t = divmod(step, N_TT)
            hp, t1b = blocks[bi]
            t1s = slice(t1b * 512, (t1b + 1) * 512)
            t2s = slice(t2t * 128, (t2t + 1) * 128)
            ps = psS.tile([128, 2, 512], f32, tag="sc")
            # row-tiled K=64 pair: even head on rows 0-63, odd on 64-127
            nc.tensor.matmul(ps[:, 0, :], kT[0:64, hp, t2s], qT[0:64, hp, t1s],
                             start=True, stop=True)
            nc.tensor.matmul(ps[:, 1, :], kT[64:128, hp, t2s], qT[64:128, hp, t1s],
                             start=True, stop=True)
            y_t = yp.tile([128, 1024], f32, tag="y")
            nc.scalar.activation(out=y_t, in_=ps.rearrange("p a b -> p (a b)"),
                                 func=AF.Exp, scale=-1.0 / SCALE, bias=mu_val)
            g_t = gp.tile([128, 1024], bf16, tag="g")
            nc.vector._custom_dve(recip_act2, out=g_t, in0=y_t, s0=RA_C0, s1=RA_C1)
            g_tiles[step] = g_t

        def emit_trail(step):
            bi, t2t = divmod(step, N_TT)
            hp, t1b = blocks[bi]
            g_t = g_tiles.pop(step)
            if t2t == 0:
                av_tiles[bi] = psV.tile([128, 2, 512], f32, tag="av", name="av")
            av = av_tiles[bi]
            h_e, h_o = 2 * hp, 2 * hp + 1
            nc.tensor.matmul(
                av[0:65, 0, :], v_aug[:, t2t, h_e * VSTR:h_e * VSTR + 65],
                g_t[:, 0:512],
                start=(t2t == 0), stop=(t2t == N_TT - 1))
            nc.tensor.matmul(
                av[0:65, 1, :], v_aug[:, t2t, h_o * VSTR:h_o * VSTR + 65],
                g_t[:, 512:1024],
                start=(t2t == 0), stop=(t2t == N_TT - 1))
            if t2t == N_TT - 1:
                emit_norm(bi)

        def emit_norm(bi):
            hp, t1b = blocks[bi]
            t1s = slice(t1b * 512, (t1b + 1) * 512)
            av = av_tiles.pop(bi)
            # one ACT copy moves values + denominator rows PSUM -> SBUF
            sb_av = nrm.tile([65, 2, 512], f32, tag="sb_av")
            nc.scalar.copy(out=sb_av, in_=av[0:65, :, :])
            # denominator rows -> partitions 0/1, one fast reciprocal
            den2 = nrm.tile([2, 512], f32, tag="den")
            nc.sync.dma_start(out=den2[0:1, :], in_=sb_av[64:65, 0, :])
            nc.sync.dma_start(out=den2[1:2, :], in_=sb_av[64:65, 1, :])
            r2 = nrm.tile([2, 512], bf16, tag="r")
            nc.vector._custom_dve(RECIPROCAL_APPROX_FAST, out=r2, in0=den2,
                                  s0=rc["s0"], s1=rc["s1"], imm2=rc["imm2"])
            nc.sync.dma_start(out=rsc_d[bi % 16], in_=r2)
            rbc = nrm.tile([64, 2, 512], bf16, tag="rbc")
            nc.sync.dma_start(out=rbc[:, 0, :],
                              in_=_broadcast_row(rsc_d[bi % 16, 0:1, :], 64))
            nc.sync.dma_start(out=rbc[:, 1, :],
                              in_=_broadcast_row(rsc_d[bi % 16, 1:2, :], 64))
            # normalize on the Pool engine (SBUF-only)
            nc.gpsimd.tensor_tensor(out=attn_sb[0:64, hp, t1s],
                                    in0=sb_av[0:64, 0, :], in1=rbc[:, 0, :],
                                    op=ALU.mult)
            tmp = nrm.tile([64, 512], bf16, tag="tmp")
            nc.gpsimd.tensor_tensor(out=tmp, in0=sb_av[0:64, 1, :],
                                    in1=rbc[:, 1, :], op=ALU.mult)
            nc.sync.dma_start(out=attn_sb[64:128, hp, t1s], in_=tmp)

        for step in range(n_steps + LAG):
            if step < n_steps:
                emit_lead(step)
            if step - LAG >= 0:
                emit_trail(step - LAG)


def _phase_c(tc, nc, mybir, attn_sb, wo_d, out_d):
    f32 = mybir.dt.float32
    bf16 = mybir.dt.bfloat16
    with (
        tc.tile_pool(name="wo", bufs=1) as wop,
        tc.tile_pool(name="ob", bufs=3) as op_,
        tc.tile_pool(name="psO", bufs=3, space="PSUM") as psO,
    ):
        wo_sb = wop.tile([128, 4, D_MODEL], bf16)
        nc.sync.dma_start(out=wo_sb, in_=wo_d.rearrange("(a p) e -> p a e", p=128))
        for et in range(2):
            es = slice(et * 512, (et + 1) * 512)
            for tt2 in range(N_TT):
                ps = psO.tile([128, 512], f32, tag="po")
                for dt in range(4):
                    nc.tensor.matmul(
                        ps,
                        attn_sb[:, dt, tt2 * 128:(tt2 + 1) * 128],
                        wo_sb[:, dt, es],
                        start=(dt == 0), stop=(dt == 3),
                    )
                o_t = op_.tile([128, 512], f32, tag="o")
                nc.scalar.copy(out=o_t, in_=ps)
                nc.sync.dma_start(
                    out=out_d[tt2 * 128:(tt2 + 1) * 128, es], in_=o_t)


_PROGRAM_CACHE = {}


def _get_program(mu_val: float):
    key = round(float(mu_val), 9)
    if key not in _PROGRAM_CACHE:
        _PROGRAM_CACHE[key] = _build_program(float(mu_val))
    return _PROGRAM_CACHE[key]


def make_core_inputs(sequence, ln_gamma, ln_beta, Wq, bq, Wk, bk, Wv, bv, Wo, bo, mu):
    """Host-side shard prep: per-core input dicts (gamma/beta folded into W/b)."""
    f = np.float32
    bf = _ml.bfloat16
    seq = np.asarray(sequence, f)
    g = np.asarray(ln_gamma, f)
    be = np.asarray(ln_beta, f)
    in_maps = []
    for c in range(N_CORES):
        b, grp = c // 2, c % 2
        blk = slice(W_LOC * grp, W_LOC * (grp + 1))
        Wqb = np.asarray(Wq, f)[blk]
        Wkb = np.asarray(Wk, f)[blk]
        Wvb = np.asarray(Wv, f)[blk]
        m = {
            "x": np.ascontiguousarray(seq[b]),
            "wqT": np.ascontiguousarray((Wqb * g[None, :]).T).astype(bf),
            "wkT": np.ascontiguousarray((Wkb * g[None, :]).T).astype(bf),
            "wvT": np.ascontiguousarray((Wvb * g[None, :]).T).astype(bf),
            "woT": np.ascontiguousarray(np.asarray(Wo, f)[:, blk].T).astype(bf),
            "bq": np.ascontiguousarray(np.asarray(bq, f)[blk] + Wqb @ be),
            "bk": np.ascontiguousarray(np.asarray(bk, f)[blk] + Wkb @ be),
            "bv": np.ascontiguousarray(np.asarray(bv, f)[blk] + Wvb @ be),
        }
        in_maps.append(m)
    return in_maps


def combine_outputs(results, bo):
    out = np.zeros((BATCH, SEQ, D_MODEL), np.float32)
    for c in range(N_CORES):
        out[c // 2] += results[c]["out"]
    out += np.asarray(bo, np.float32)[None, None, :]
    return out


def kernel(sequence, ln_gamma, ln_beta, Wq, bq, Wk, bk, Wv, bv, Wo, bo, mu,
           _trace=False):
    from concourse.bass_utils import run_bass_kernel_spmd

    mu_val = float(np.asarray(mu).reshape(-1)[0])
    nc = _get_program(mu_val)
    in_maps = make_core_inputs(sequence, ln_gamma, ln_beta, Wq, bq, Wk, bk,
                               Wv, bv, Wo, bo, mu)
    res = run_bass_kernel_spmd(nc, in_maps, list(range(N_CORES)), trace=_trace)
    out = combine_outputs(res.results, bo)
    if _trace:
        kernel.last_results = res
    return out


# revision 10
# speedup vs baseline: 1.2716x; 1.2357x over previous
"""Trainium2 Bass kernel for LayerNorm + multi-head attention (B=4, S=2048,
D=1024, H=16) with sigmoid(s-mu)*exp(s) row-normalized attention weights.

Sharding: 8 cores = 4 batches x 2 head-groups (8 heads each). Each core
computes LN + its head-group's q/k/v projections + attention + a partial
output projection; the host sums the two partials per batch and adds bo.

Device-side identities:
  p_i = g(z_i) / sum_j g(z_j),  z = s - mu,  g(z) = e^z * sigmoid(z)
  1 / g(z) = y * (1 + y)  with  y = e^{-z}
so per score element: ONE ScalarE exp + ONE fused custom-DVE op computing
1/(x + x^2) (bitwise-not reciprocal seed + one Newton step).

v3 layout/scheduling:
  - all matmul operands bf16; ONE 8-bank PSUM pool for the whole program
    (tags "sc" + "av", 2 bufs each, [128,2,512] f32 = 2 banks per tile)
  - prefix: per token tile LN -> PE transposes -> v projection, fully
    interleaved; only the hp=0 q/k projections gate the start of attention
  - score matmuls for the even/odd head of a pair are K=64 row-tiled into
    the two banks of one "sc" tile; ONE exp + ONE recip-act2 call covers
    both parities (1024 cols)
  - AV matmuls trail the score matmuls by LAG k-tiles so the in-order PE
    queue never stalls on the DVE; q/k projections for hp+1 are injected
    into phase B's PE slack; phase C runs per 512-token block as soon as
    the last head pair's normalization lands
  - normalization: one ACT copy moves values+denominator rows to SBUF,
    one DVE fast-reciprocal, DMA broadcast via a DRAM bounce row, final
    multiplies on the (otherwise idle) Pool/GpSimd engine
"""

import sys

if "/opt/trn_rl_repo" not in sys.path:
    sys.path.insert(0, "/opt/trn_rl_repo")

import numpy as np
import ml_dtypes as _ml

D_MODEL = 1024
N_HEADS = 16
HEAD_DIM = 64
SEQ = 2048
BATCH = 4
N_CORES = 8
EPS = 1e-6
SCALE = float(HEAD_DIM) ** 0.5  # 8.0

# Per-core partitioning
H_LOC = 8          # heads per core
W_LOC = H_LOC * HEAD_DIM  # 512 local projection width
N_DT = D_MODEL // 128     # 8 d-tiles
N_TT = SEQ // 128         # 16 token tiles (t2 direction)
VSTR = 66                 # per-head stride in v_aug ([64 v | 1 ones | 1 pad])
LAG = 2                   # AV matmuls trail score matmuls by LAG k-tiles

# Custom-DVE fused reciprocal-of-act2 constants.
RA_C0 = -0.234
RA_C1 = 2.0


def _get_recip_act2_op():
    """Register (once) and return the custom DVE op: out = 1/(x + x^2)."""
    import concourse.dve_ops as dve_ops

    if hasattr(dve_ops, "RECIP_ACT2_ANT"):
        return dve_ops.RECIP_ACT2_ANT

    from concourse.dve_spec import Spec, Src0, C0, C1, Bin, AluOp, sq, lower, _has_src1
    from concourse.dve_uop import DveOpSpec

    _w = sq(Src0) + Src0
    _nw = Bin(AluOp.BITWISE_NOT, _w, _w)
    _y0 = _nw * C0
    _body = _y0 * (C1 - _w * _y0)

    def _ref(in0, in1, s0, s1, imm2):
        x = np.asarray(in0).astype(np.float32)
        w = (x + x * x).astype(np.float32)
        nw = (~w.view(np.int32)).view(np.float32)
        if isinstance(s0, np.ndarray):
            s0 = s0.astype(np.float32)
        if isinstance(s1, np.ndarray):
            s1 = s1.astype(np.float32)
        y0 = (nw * np.float32(s0) if not isinstance(s0, np.ndarray) else nw * s0).astype(np.float32)
        c1 = np.float32(s1) if not isinstance(s1, np.ndarray) else s1
        return (y0 * (c1 - w * y0)).astype(np.float32)

    spec = Spec(body=_body, reference=_ref)
    name = "RECIP_ACT2_ANT"
    row = max(dve_ops._SUB_OPCODE_FOR_NAME.values()) + 1
    assert row < 0x20
    dve_ops._SUB_OPCODE_FOR_NAME[name] = row
    shas = {}
    for ver in ("v3", "v4"):
        compiled = DveOpSpec(
            name=name, opcode=row, uops=lower(spec, ver=ver), rd1_en=_has_src1(spec)
        )
        shas[ver] = compiled.sha(ver)
    op = dve_ops.DveOp(name, spec, subdim=False, uops_sha=shas)
    dve_ops.OPS.append(op)
    dve_ops.CUSTOM_DVE_SPECS[name] = spec
    dve_ops.RECIP_ACT2_ANT = op
    return op


def _broadcast_ap(ap, parts):
    """Partition-broadcast a 1-D DRAM AP of shape [N] to [parts, N]."""
    import concourse.bass as bass

    steps = [list(p) for p in ap.ap]
    return bass.AP(tensor=ap.tensor, offset=ap.offset, ap=[[0, parts]] + steps)


def _broadcast_row(ap2d, parts):
    """Partition-broadcast a [1, N] DRAM AP to [parts, N]."""
    import concourse.bass as bass

    steps = [list(p) for p in ap2d.ap[1:]]
    return bass.AP(tensor=ap2d.tensor, offset=ap2d.offset, ap=[[0, parts]] + steps)


def _build_program(mu_val: float):
    import concourse.mybir as mybir
    import concourse.tile as tile
    from concourse import bacc
    from concourse.masks import make_identity
    from concourse.dve_ops import RECIPROCAL_APPROX_FAST, RECIP_APPROX_FAST_CONSTS

    recip_act2 = _get_recip_act2_op()

    f32 = mybir.dt.float32
    bf16 = mybir.dt.bfloat16
    AF = mybir.ActivationFunctionType
    ALU = mybir.AluOpType

    nc = bacc.Bacc("TRN2", target_bir_lowering=False, debug=False,
                   num_devices=N_CORES)

    x_d = nc.dram_tensor("x", [SEQ, D_MODEL], f32, kind="ExternalInput").ap()
    wq_d = nc.dram_tensor("wqT", [D_MODEL, W_LOC], bf16, kind="ExternalInput").ap()
    wk_d = nc.dram_tensor("wkT", [D_MODEL, W_LOC], bf16, kind="ExternalInput").ap()
    wv_d = nc.dram_tensor("wvT", [D_MODEL, W_LOC], bf16, kind="ExternalInput").ap()
    wo_d = nc.dram_tensor("woT", [W_LOC, D_MODEL], bf16, kind="ExternalInput").ap()
    bq_d = nc.dram_tensor("bq", [W_LOC], f32, kind="ExternalInput").ap()
    bk_d = nc.dram_tensor("bk", [W_LOC], f32, kind="ExternalInput").ap()
    bv_d = nc.dram_tensor("bv", [W_LOC], f32, kind="ExternalInput").ap()
    out_d = nc.dram_tensor("out", [SEQ, D_MODEL], f32, kind="ExternalOutput").ap()
    # DRAM bounce rows for the per-block reciprocals (DMA partition-broadcast
    # needs a DRAM source); one row per (hp, t1b) block.
    rsc_d = nc.dram_tensor("rscratch", [16, 2, 512], bf16, kind="Internal").ap()

    rc = RECIP_APPROX_FAST_CONSTS

    with tile.TileContext(nc) as tc:
        with (
            tc.tile_pool(name="consts", bufs=1) as consts,
            tc.tile_pool(name="qkv", bufs=1) as qkv_pool,
            tc.tile_pool(name="xp", bufs=3) as xp,
            tc.tile_pool(name="sp", bufs=4) as sp,
            tc.tile_pool(name="yb", bufs=3) as yp,
            tc.tile_pool(name="gb", bufs=LAG + 2) as gp,
            tc.tile_pool(name="nrm", bufs=2) as nrm,
            tc.tile_pool(name="ob", bufs=3) as op_,
            tc.tile_pool(name="ps8", bufs=2, space="PSUM") as ps8,
        ):
            ident = consts.tile([128, 128], bf16)
            make_identity(nc, ident)
            eps_sb = consts.tile([128, 1], f32)
            nc.vector.memset(eps_sb, EPS)
            bq_sb = consts.tile([128, 4], f32)
            nc.sync.dma_start(out=bq_sb, in_=bq_d.rearrange("(a p) -> p a", p=128))
            bk_sb = consts.tile([128, 4], f32)
            nc.sync.dma_start(out=bk_sb, in_=bk_d.rearrange("(a p) -> p a", p=128))
            bv_bc = consts.tile([128, W_LOC], f32)
            nc.sync.dma_start(out=bv_bc, in_=_broadcast_ap(bv_d, 128))

            # persistent activations (all bf16)
            qT = qkv_pool.tile([128, 4, SEQ], bf16)   # [pair-dim, hp, t]
            kT = qkv_pool.tile([128, 4, SEQ], bf16)
            v_aug = qkv_pool.tile([128, N_TT, H_LOC * VSTR], bf16)
            attn_sb = qkv_pool.tile([128, 4, SEQ], bf16)
            nc.vector.memset(
                v_aug.rearrange("p t (h c) -> p t h c", c=VSTR)[:, :, :, 64:66], 1.0
            )

            wq_sb = qkv_pool.tile([128, N_DT, W_LOC], bf16)
            nc.sync.dma_start(out=wq_sb, in_=wq_d.rearrange("(a p) j -> p a j", p=128))
            wk_sb = qkv_pool.tile([128, N_DT, W_LOC], bf16)
            nc.sync.dma_start(out=wk_sb, in_=wk_d.rearrange("(a p) j -> p a j", p=128))
            wv_sb = qkv_pool.tile([128, N_DT, W_LOC], bf16)
            nc.sync.dma_start(out=wv_sb, in_=wv_d.rearrange("(a p) j -> p a j", p=128))
            wo_sb = qkv_pool.tile([128, 4, D_MODEL], bf16)
            nc.sync.dma_start(out=wo_sb, in_=wo_d.rearrange("(a p) e -> p a e", p=128))
            xT = qkv_pool.tile([128, N_DT, SEQ], bf16)

            # ---------------- prefix: LN + transpose + v-proj, interleaved --
            for tt in range(N_TT):
                x_t = xp.tile([128, D_MODEL], f32, tag="x")
                nc.sync.dma_start(out=x_t, in_=x_d[tt * 128:(tt + 1) * 128, :])
                stats = sp.tile([128, 2, 6], f32, tag="bn")
                nc.vector.bn_stats(out=stats[:, 0, :], in_=x_t[:, 0:512])
                nc.vector.bn_stats(out=stats[:, 1, :], in_=x_t[:, 512:1024])
                mv = sp.tile([128, 2], f32, tag="mv")
                nc.vector.bn_aggr(out=mv, in_=stats)
                rstd = sp.tile([128, 1], f32, tag="rstd")
                nc.scalar.activation(out=rstd, in_=mv[:, 1:2], func=AF.Sqrt,
                                     bias=eps_sb, scale=1.0)
                nc.vector.reciprocal(out=rstd, in_=rstd)
                nmr = sp.tile([128, 1], f32, tag="nmr")
                nc.vector.tensor_scalar(out=nmr, in0=mv[:, 0:1], scalar1=rstd,
                                        scalar2=-1.0, op0=ALU.mult, op1=ALU.mult)
                xs_t = xp.tile([128, D_MODEL], bf16, tag="xs")
                nc.vector.tensor_scalar(out=xs_t, in0=x_t, scalar1=rstd,
                                        scalar2=nmr, op0=ALU.mult, op1=ALU.add)
                for d in range(N_DT):
                    ps_tr = ps8.tile([128, 128], bf16, tag="av", name="ps_tr")
                    nc.tensor.transpose(ps_tr, xs_t[:, d * 128:(d + 1) * 128], ident)
                    if d % 2 == 0:
                        nc.scalar.copy(
                            out=xT[:, d, tt * 128:(tt + 1) * 128], in_=ps_tr)
                    else:
                        nc.vector.tensor_copy(
                            out=xT[:, d, tt * 128:(tt + 1) * 128], in_=ps_tr)
                # v projection for this token tile
                ps_v = ps8.tile([128, 2, 512], f32, tag="sc", name="ps_v")
                for d in range(N_DT):
                    nc.tensor.matmul(
                        ps_v[:, 0, :],
                        xT[:, d, tt * 128:(tt + 1) * 128],
                        wv_sb[:, d, :],
                        start=(d == 0), stop=(d == N_DT - 1),
                    )
                v_view = v_aug.rearrange("p t (h c) -> p t h c", c=VSTR)[:, tt, :, 0:64]
                nc.vector.tensor_tensor(
                    out=v_view,
                    in0=ps_v[:, 0, :].rearrange("p (h c) -> p h c", c=64),
                    in1=bv_bc.rearrange("p (h c) -> p h c", c=64), op=ALU.add)

            def emit_qk_proj(hp, which, cb):
                """One q-or-k projection output tile: [128, 512] for (hp, chunk)."""
                w_sb, b_sb, dstT = (
                    (wq_sb, bq_sb, qT) if which == 0 else (wk_sb, bk_sb, kT))
                ps_p = ps8.tile([128, 2, 512], f32, tag="sc", name="ps_p")
                for d in range(N_DT):
                    nc.tensor.matmul(
                        ps_p[:, 0, :],
                        w_sb[:, d, hp * 128:(hp + 1) * 128],
                        xT[:, d, cb * 512:(cb + 1) * 512],
                        start=(d == 0), stop=(d == N_DT - 1),
                    )
                nc.scalar.activation(
                    out=dstT[:, hp, cb * 512:(cb + 1) * 512], in_=ps_p[:, 0, :],
                    func=AF.Identity, bias=b_sb[:, hp:hp + 1], scale=1.0)

            for which in range(2):
                for cb in range(4):
                    emit_qk_proj(0, which, cb)

            # ---------------- phase B + injected proj/C ---------------------
            _phase_b(tc, nc, mybir, qT, kT, v_aug, attn_sb, rsc_d, mu_val,
                     recip_act2, RECIPROCAL_APPROX_FAST, rc,
                     ps8, yp, gp, nrm, op_, emit_qk_proj, wo_sb, out_d)

    nc.compile()
    return nc


def _phase_b(tc, nc, mybir, qT, kT, v_aug, attn_sb, rsc_d, mu_val,
             recip_act2, RECIPROCAL_APPROX_FAST, rc,
             ps8, yp, gp, nrm, op_, emit_qk_proj, wo_sb, out_d):
    f32 = mybir.dt.float32
    bf16 = mybir.dt.bfloat16
    AF = mybir.ActivationFunctionType
    ALU = mybir.AluOpType

    blocks = [(hp, t1b) for hp in range(4) for t1b in range(4)]
    n_steps = len(blocks) * N_TT

    g_tiles = {}
    av_tiles = {}

    def emit_lead(step):
        bi, t2t = divmod(step, N_TT)
        hp, t1b = blocks[bi]
        t1s = slice(t1b * 512, (t1b + 1) * 512)
        t2s = slice(t2t * 128, (t2t + 1) * 128)
        ps = ps8.tile([128, 2, 512], f32, tag="sc", name="ps_s")
        nc.tensor.matmul(ps[:, 0, :], kT[0:64, hp, t2s], qT[0:64, hp, t1s],
                         start=True, stop=True)
        nc.tensor.matmul(ps[:, 1, :], kT[64:128, hp, t2s], qT[64:128, hp, t1s],
                         start=True, stop=True)
        y_t = yp.tile([128, 1024], bf16, tag="y")
        nc.scalar.activation(out=y_t, in_=ps.rearrange("p a b -> p (a b)"),
                             func=AF.Exp, scale=-1.0 / SCALE, bias=mu_val)
        g_t = gp.tile([128, 1024], bf16, tag="g")
        nc.vector._custom_dve(recip_act2, out=g_t, in0=y_t, s0=RA_C0, s1=RA_C1)
        g_tiles[step] = g_t
        # inject q/k projections for the next head pair into PE slack
        if hp < 3 and t2t in (5, 11):
            i = 2 * t1b + (0 if t2t == 5 else 1)
            emit_qk_proj(hp + 1, 0 if i < 4 else 1, i % 4)

    def emit_trail(step):
        bi, t2t = divmod(step, N_TT)
        hp, t1b = blocks[bi]
        g_t = g_tiles.pop(step)
        if t2t == 0:
            av_tiles[bi] = ps8.tile([128, 2, 512], f32, tag="av", name="av")
        av = av_tiles[bi]
        h_e, h_o = 2 * hp, 2 * hp + 1
        nc.tensor.matmul(
            av[0:65, 0, :], v_aug[:, t2t, h_e * VSTR:h_e * VSTR + 65],
            g_t[:, 0:512],
            start=(t2t == 0), stop=(t2t == N_TT - 1))
        nc.tensor.matmul(
            av[0:65, 1, :], v_aug[:, t2t, h_o * VSTR:h_o * VSTR + 65],
            g_t[:, 512:1024],
            start=(t2t == 0), stop=(t2t == N_TT - 1))
        if t2t == N_TT - 1:
            emit_norm(bi)
            if blocks[bi][0] == 3:
                emit_c(blocks[bi][1])

    def emit_norm(bi):
        hp, t1b = blocks[bi]
        t1s = slice(t1b * 512, (t1b + 1) * 512)
        av = av_tiles.pop(bi)
        sb_av = nrm.tile([65, 2, 512], f32, tag="sb_av")
        nc.scalar.copy(out=sb_av, in_=av[0:65, :, :])
        den2 = nrm.tile([2, 512], f32, tag="den")
        nc.sync.dma_start(out=den2[0:1, :], in_=sb_av[64:65, 0, :])
        nc.sync.dma_start(out=den2[1:2, :], in_=sb_av[64:65, 1, :])
        r2 = nrm.tile([2, 512], bf16, tag="r")
        nc.vector._custom_dve(RECIPROCAL_APPROX_FAST, out=r2, in0=den2,
                              s0=rc["s0"], s1=rc["s1"], imm2=rc["imm2"])
        nc.sync.dma_start(out=rsc_d[bi], in_=r2)
        rbc = nrm.tile([64, 2, 512], bf16, tag="rbc")
        nc.sync.dma_start(out=rbc[:, 0, :],
                          in_=_broadcast_row(rsc_d[bi, 0:1, :], 64))
        nc.sync.dma_start(out=rbc[:, 1, :],
                          in_=_broadcast_row(rsc_d[bi, 1:2, :], 64))
        nc.gpsimd.tensor_tensor(out=attn_sb[0:64, hp, t1s],
                                in0=sb_av[0:64, 0, :], in1=rbc[:, 0, :],
                                op=ALU.mult)
        tmp = nrm.tile([64, 512], bf16, tag="tmp")
        nc.gpsimd.tensor_tensor(out=tmp, in0=sb_av[0:64, 1, :],
                                in1=rbc[:, 1, :], op=ALU.mult)
        nc.sync.dma_start(out=attn_sb[64:128, hp, t1s], in_=tmp)

    def emit_c(t1b):
        """Output projection for one 512-token block (needs all 4 head pairs)."""
        for tt2 in range(4 * t1b, 4 * t1b + 4):
            for et in range(2):
                es = slice(et * 512, (et + 1) * 512)
                ps_o = ps8.tile([128, 2, 512], f32, tag="sc", name="ps_o")
                for dt in range(4):
                    nc.tensor.matmul(
                        ps_o[:, 0, :],
                        attn_sb[:, dt, tt2 * 128:(tt2 + 1) * 128],
                        wo_sb[:, dt, es],
                        start=(dt == 0), stop=(dt == 3),
                    )
                o_t = op_.tile([128, 512], f32, tag="o")
                nc.scalar.copy(out=o_t, in_=ps_o[:, 0, :])
                nc.sync.dma_start(
                    out=out_d[tt2 * 128:(tt2 + 1) * 128, es], in_=o_t)

    for step in range(n_steps + LAG):
        if step < n_steps:
            emit_lead(step)
        if step - LAG >= 0:
            emit_trail(step - LAG)


_PROGRAM_CACHE = {}


def _get_program(mu_val: float):
    key = round(float(mu_val), 9)
    if key not in _PROGRAM_CACHE:
        _PROGRAM_CACHE[key] = _build_program(float(mu_val))
    return _PROGRAM_CACHE[key]


def make_core_inputs(sequence, ln_gamma, ln_beta, Wq, bq, Wk, bk, Wv, bv, Wo, bo, mu):
    """Host-side shard prep: per-core input dicts (gamma/beta folded into W/b)."""
    f = np.float32
    bf = _ml.bfloat16
    seq = np.asarray(sequence, f)
    g = np.asarray(ln_gamma, f)
    be = np.asarray(ln_beta, f)
    in_maps = []
    for c in range(N_CORES):
        b, grp = c // 2, c % 2
        blk = slice(W_LOC * grp, W_LOC * (grp + 1))
        Wqb = np.asarray(Wq, f)[blk]
        Wkb = np.asarray(Wk, f)[blk]
        Wvb = np.asarray(Wv, f)[blk]
        m = {
            "x": np.ascontiguousarray(seq[b]),
            "wqT": np.ascontiguousarray((Wqb * g[None, :]).T).astype(bf),
            "wkT": np.ascontiguousarray((Wkb * g[None, :]).T).astype(bf),
            "wvT": np.ascontiguousarray((Wvb * g[None, :]).T).astype(bf),
            "woT": np.ascontiguousarray(np.asarray(Wo, f)[:, blk].T).astype(bf),
            "bq": np.ascontiguousarray(np.asarray(bq, f)[blk] + Wqb @ be),
            "bk": np.ascontiguousarray(np.asarray(bk, f)[blk] + Wkb @ be),
            "bv": np.ascontiguousarray(np.asarray(bv, f)[blk] + Wvb @ be),
        }
        in_maps.append(m)
    return in_maps


def combine_outputs(results, bo):
    out = np.zeros((BATCH, SEQ, D_MODEL), np.float32)
    for c in range(N_CORES):
        out[c // 2] += results[c]["out"]
    out += np.asarray(bo, np.float32)[None, None, :]
    return out


def kernel(sequence, ln_gamma, ln_beta, Wq, bq, Wk, bk, Wv, bv, Wo, bo, mu,
           _trace=False):
    from concourse.bass_utils import run_bass_kernel_spmd

    mu_val = float(np.asarray(mu).reshape(-1)[0])
    nc = _get_program(mu_val)
    in_maps = make_core_inputs(sequence, ln_gamma, ln_beta, Wq, bq, Wk, bk,
                               Wv, bv, Wo, bo, mu)
    res = run_bass_kernel_spmd(nc, in_maps, list(range(N_CORES)), trace=_trace)
    out = combine_outputs(res.results, bo)
    if _trace:
        kernel.last_results = res
    return out


# revision 11
# speedup vs baseline: 1.3040x; 1.0254x over previous
"""Trainium2 Bass kernel for LayerNorm + multi-head attention (B=4, S=2048,
D=1024, H=16) with sigmoid(s-mu)*exp(s) row-normalized attention weights.

Sharding: 8 cores = 4 batches x 2 head-groups (8 heads each). Each core
computes LN + its head-group's q/k/v projections + attention + a partial
output projection; the host sums the two partials per batch and adds bo.

Device-side identities:
  p_i = g(z_i) / sum_j g(z_j),  z = s - mu,  g(z) = e^z * sigmoid(z)
  1 / g(z) = y * (1 + y)  with  y = e^{-z}
so per score element: ONE ScalarE exp + ONE fused custom-DVE op computing
1/(x + x^2) (bitwise-not reciprocal seed + one Newton step).

v3 layout/scheduling:
  - all matmul operands bf16; ONE 8-bank PSUM pool for the whole program
    (tags "sc" + "av", 2 bufs each, [128,2,512] f32 = 2 banks per tile)
  - prefix: per token tile LN -> PE transposes -> v projection, fully
    interleaved; only the hp=0 q/k projections gate the start of attention
  - score matmuls for the even/odd head of a pair are K=64 row-tiled into
    the two banks of one "sc" tile; ONE exp + ONE recip-act2 call covers
    both parities (1024 cols)
  - AV matmuls trail the score matmuls by LAG k-tiles so the in-order PE
    queue never stalls on the DVE; q/k projections for hp+1 are injected
    into phase B's PE slack; phase C runs per 512-token block as soon as
    the last head pair's normalization lands
  - normalization: one ACT copy moves values+denominator rows to SBUF,
    one DVE fast-reciprocal, DMA broadcast via a DRAM bounce row, final
    multiplies on the (otherwise idle) Pool/GpSimd engine
"""

import sys

if "/opt/trn_rl_repo" not in sys.path:
    sys.path.insert(0, "/opt/trn_rl_repo")

import numpy as np
import ml_dtypes as _ml

D_MODEL = 1024
N_HEADS = 16
HEAD_DIM = 64
SEQ = 2048
BATCH = 4
N_CORES = 8
EPS = 1e-6
SCALE = float(HEAD_DIM) ** 0.5  # 8.0

# Per-core partitioning
H_LOC = 8          # heads per core
W_LOC = H_LOC * HEAD_DIM  # 512 local projection width
N_DT = D_MODEL // 128     # 8 d-tiles
N_TT = SEQ // 128         # 16 token tiles (t2 direction)
VSTR = 66                 # per-head stride in v_aug ([64 v | 1 ones | 1 pad])
LAG = 2                   # AV matmuls trail score matmuls by LAG k-tiles

# Custom-DVE fused reciprocal-of-act2 constants.
RA_C0 = -0.234
RA_C1 = 2.0


def _get_recip_act2_op():
    """Register (once) and return the custom DVE op: out = 1/(x + x^2)."""
    import concourse.dve_ops as dve_ops

    if hasattr(dve_ops, "RECIP_ACT2_ANT"):
        return dve_ops.RECIP_ACT2_ANT

    from concourse.dve_spec import Spec, Src0, C0, C1, Bin, AluOp, sq, lower, _has_src1
    from concourse.dve_uop import DveOpSpec

    _w = sq(Src0) + Src0
    _nw = Bin(AluOp.BITWISE_NOT, _w, _w)
    _y0 = _nw * C0
    _body = _y0 * (C1 - _w * _y0)

    def _ref(in0, in1, s0, s1, imm2):
        x = np.asarray(in0).astype(np.float32)
        w = (x + x * x).astype(np.float32)
        nw = (~w.view(np.int32)).view(np.float32)
        if isinstance(s0, np.ndarray):
            s0 = s0.astype(np.float32)
        if isinstance(s1, np.ndarray):
            s1 = s1.astype(np.float32)
        y0 = (nw * np.float32(s0) if not isinstance(s0, np.ndarray) else nw * s0).astype(np.float32)
        c1 = np.float32(s1) if not isinstance(s1, np.ndarray) else s1
        return (y0 * (c1 - w * y0)).astype(np.float32)

    spec = Spec(body=_body, reference=_ref)
    name = "RECIP_ACT2_ANT"
    row = max(dve_ops._SUB_OPCODE_FOR_NAME.values()) + 1
    assert row < 0x20
    dve_ops._SUB_OPCODE_FOR_NAME[name] = row
    shas = {}
    for ver in ("v3", "v4"):
        compiled = DveOpSpec(
            name=name, opcode=row, uops=lower(spec, ver=ver), rd1_en=_has_src1(spec)
        )
        shas[ver] = compiled.sha(ver)
    op = dve_ops.DveOp(name, spec, subdim=False, uops_sha=shas)
    dve_ops.OPS.append(op)
    dve_ops.CUSTOM_DVE_SPECS[name] = spec
    dve_ops.RECIP_ACT2_ANT = op
    return op


def _broadcast_ap(ap, parts):
    """Partition-broadcast a 1-D DRAM AP of shape [N] to [parts, N]."""
    import concourse.bass as bass

    steps = [list(p) for p in ap.ap]
    return bass.AP(tensor=ap.tensor, offset=ap.offset, ap=[[0, parts]] + steps)


def _broadcast_row(ap2d, parts):
    """Partition-broadcast a [1, N] DRAM AP to [parts, N]."""
    import concourse.bass as bass

    steps = [list(p) for p in ap2d.ap[1:]]
    return bass.AP(tensor=ap2d.tensor, offset=ap2d.offset, ap=[[0, parts]] + steps)


def _build_program(mu_val: float):
    import concourse.mybir as mybir
    import concourse.tile as tile
    from concourse import bacc
    from concourse.masks import make_identity
    from concourse.dve_ops import RECIPROCAL_APPROX_FAST, RECIP_APPROX_FAST_CONSTS

    recip_act2 = _get_recip_act2_op()

    f32 = mybir.dt.float32
    bf16 = mybir.dt.bfloat16
    AF = mybir.ActivationFunctionType
    ALU = mybir.AluOpType

    nc = bacc.Bacc("TRN2", target_bir_lowering=False, debug=False,
                   num_devices=N_CORES)

    x_d = nc.dram_tensor("x", [SEQ, D_MODEL], f32, kind="ExternalInput").ap()
    wq_d = nc.dram_tensor("wqT", [D_MODEL, W_LOC], bf16, kind="ExternalInput").ap()
    wk_d = nc.dram_tensor("wkT", [D_MODEL, W_LOC], bf16, kind="ExternalInput").ap()
    wv_d = nc.dram_tensor("wvT", [D_MODEL, W_LOC], bf16, kind="ExternalInput").ap()
    wo_d = nc.dram_tensor("woT", [W_LOC, D_MODEL], bf16, kind="ExternalInput").ap()
    bq_d = nc.dram_tensor("bq", [W_LOC], f32, kind="ExternalInput").ap()
    bk_d = nc.dram_tensor("bk", [W_LOC], f32, kind="ExternalInput").ap()
    bv_d = nc.dram_tensor("bv", [W_LOC], f32, kind="ExternalInput").ap()
    out_d = nc.dram_tensor("out", [SEQ, D_MODEL], f32, kind="ExternalOutput").ap()
    # DRAM bounce rows for the per-block reciprocals (DMA partition-broadcast
    # needs a DRAM source); one row per (hp, t1b) block.
    rsc_d = nc.dram_tensor("rscratch", [16, 2, 512], bf16, kind="Internal").ap()

    rc = RECIP_APPROX_FAST_CONSTS

    with tile.TileContext(nc) as tc:
        with (
            tc.tile_pool(name="consts", bufs=1) as consts,
            tc.tile_pool(name="qkv", bufs=1) as qkv_pool,
            tc.tile_pool(name="xp", bufs=3) as xp,
            tc.tile_pool(name="sp", bufs=4) as sp,
            tc.tile_pool(name="yb", bufs=3) as yp,
            tc.tile_pool(name="gb", bufs=LAG + 2) as gp,
            tc.tile_pool(name="nrm", bufs=2) as nrm,
            tc.tile_pool(name="ob", bufs=3) as op_,
            tc.tile_pool(name="ps8", bufs=2, space="PSUM") as ps8,
        ):
            ident = consts.tile([128, 128], bf16)
            make_identity(nc, ident)
            eps_sb = consts.tile([128, 1], f32)
            nc.vector.memset(eps_sb, EPS)
            bq_sb = consts.tile([128, 4], f32)
            nc.sync.dma_start(out=bq_sb, in_=bq_d.rearrange("(a p) -> p a", p=128))
            bk_sb = consts.tile([128, 4], f32)
            nc.sync.dma_start(out=bk_sb, in_=bk_d.rearrange("(a p) -> p a", p=128))
            bv_bc = consts.tile([128, W_LOC], f32)
            nc.sync.dma_start(out=bv_bc, in_=_broadcast_ap(bv_d, 128))

            # persistent activations (all bf16)
            qT = qkv_pool.tile([128, 4, SEQ], bf16)   # [pair-dim, hp, t]
            kT = qkv_pool.tile([128, 4, SEQ], bf16)
            v_aug = qkv_pool.tile([128, N_TT, H_LOC * VSTR], bf16)
            attn_sb = qkv_pool.tile([128, 4, SEQ], bf16)
            nc.vector.memset(
                v_aug.rearrange("p t (h c) -> p t h c", c=VSTR)[:, :, :, 64:66], 1.0
            )

            wq_sb = qkv_pool.tile([128, N_DT, W_LOC], bf16)
            nc.sync.dma_start(out=wq_sb, in_=wq_d.rearrange("(a p) j -> p a j", p=128))
            wk_sb = qkv_pool.tile([128, N_DT, W_LOC], bf16)
            nc.sync.dma_start(out=wk_sb, in_=wk_d.rearrange("(a p) j -> p a j", p=128))
            wv_sb = qkv_pool.tile([128, N_DT, W_LOC], bf16)
            nc.sync.dma_start(out=wv_sb, in_=wv_d.rearrange("(a p) j -> p a j", p=128))
            wo_sb = qkv_pool.tile([128, 4, D_MODEL], bf16)
            nc.sync.dma_start(out=wo_sb, in_=wo_d.rearrange("(a p) e -> p a e", p=128))
            xT = qkv_pool.tile([128, N_DT, SEQ], bf16)

            # ---------------- prefix: LN + transpose + v-proj, interleaved --
            for tt in range(N_TT):
                x_t = xp.tile([128, D_MODEL], f32, tag="x")
                nc.sync.dma_start(out=x_t, in_=x_d[tt * 128:(tt + 1) * 128, :])
                stats = sp.tile([128, 2, 6], f32, tag="bn")
                nc.vector.bn_stats(out=stats[:, 0, :], in_=x_t[:, 0:512])
                nc.vector.bn_stats(out=stats[:, 1, :], in_=x_t[:, 512:1024])
                mv = sp.tile([128, 2], f32, tag="mv")
                nc.vector.bn_aggr(out=mv, in_=stats)
                rstd = sp.tile([128, 1], f32, tag="rstd")
                nc.scalar.activation(out=rstd, in_=mv[:, 1:2], func=AF.Sqrt,
                                     bias=eps_sb, scale=1.0)
                nc.vector.reciprocal(out=rstd, in_=rstd)
                nmr = sp.tile([128, 1], f32, tag="nmr")
                nc.vector.tensor_scalar(out=nmr, in0=mv[:, 0:1], scalar1=rstd,
                                        scalar2=-1.0, op0=ALU.mult, op1=ALU.mult)
                xs_t = xp.tile([128, D_MODEL], bf16, tag="xs")
                nc.scalar.activation(out=xs_t, in_=x_t, func=AF.Identity,
                                     scale=rstd, bias=nmr)
                for d in range(N_DT):
                    ps_tr = ps8.tile([128, 128], bf16, tag="av", name="ps_tr")
                    nc.tensor.transpose(ps_tr, xs_t[:, d * 128:(d + 1) * 128], ident)
                    if d % 2 == 0:
                        nc.scalar.copy(
                            out=xT[:, d, tt * 128:(tt + 1) * 128], in_=ps_tr)
                    else:
                        nc.vector.tensor_copy(
                            out=xT[:, d, tt * 128:(tt + 1) * 128], in_=ps_tr)
                # v projection for this token tile
                ps_v = ps8.tile([128, 2, 512], f32, tag="sc", name="ps_v")
                for d in range(N_DT):
                    nc.tensor.matmul(
                        ps_v[:, 0, :],
                        xT[:, d, tt * 128:(tt + 1) * 128],
                        wv_sb[:, d, :],
                        start=(d == 0), stop=(d == N_DT - 1),
                    )
                v_view = v_aug.rearrange("p t (h c) -> p t h c", c=VSTR)[:, tt, :, 0:64]
                nc.vector.tensor_tensor(
                    out=v_view,
                    in0=ps_v[:, 0, :].rearrange("p (h c) -> p h c", c=64),
                    in1=bv_bc.rearrange("p (h c) -> p h c", c=64), op=ALU.add)

            def emit_qk_proj(hp, which, cb):
                """One q-or-k projection output tile: [128, 512] for (hp, chunk)."""
                w_sb, b_sb, dstT = (
                    (wq_sb, bq_sb, qT) if which == 0 else (wk_sb, bk_sb, kT))
                ps_p = ps8.tile([128, 2, 512], f32, tag="sc", name="ps_p")
                for d in range(N_DT):
                    nc.tensor.matmul(
                        ps_p[:, 0, :],
                        w_sb[:, d, hp * 128:(hp + 1) * 128],
                        xT[:, d, cb * 512:(cb + 1) * 512],
                        start=(d == 0), stop=(d == N_DT - 1),
                    )
                nc.scalar.activation(
                    out=dstT[:, hp, cb * 512:(cb + 1) * 512], in_=ps_p[:, 0, :],
                    func=AF.Identity, bias=b_sb[:, hp:hp + 1], scale=1.0)

            for hp in range(4):
                for which in range(2):
                    for cb in range(4):
                        emit_qk_proj(hp, which, cb)

            # ---------------- phase B + injected proj/C ---------------------
            _phase_b(tc, nc, mybir, qT, kT, v_aug, attn_sb, rsc_d, mu_val,
                     recip_act2, RECIPROCAL_APPROX_FAST, rc,
                     ps8, yp, gp, nrm, op_, emit_qk_proj, wo_sb, out_d)

    nc.compile()
    return nc


def _phase_b(tc, nc, mybir, qT, kT, v_aug, attn_sb, rsc_d, mu_val,
             recip_act2, RECIPROCAL_APPROX_FAST, rc,
             ps8, yp, gp, nrm, op_, emit_qk_proj, wo_sb, out_d):
    f32 = mybir.dt.float32
    bf16 = mybir.dt.bfloat16
    AF = mybir.ActivationFunctionType
    ALU = mybir.AluOpType

    blocks = [(hp, t1b) for hp in range(4) for t1b in range(4)]
    n_steps = len(blocks) * N_TT

    g_tiles = {}
    av_tiles = {}

    def emit_lead(step):
        bi, t2t = divmod(step, N_TT)
        hp, t1b = blocks[bi]
        t1s = slice(t1b * 512, (t1b + 1) * 512)
        t2s = slice(t2t * 128, (t2t + 1) * 128)
        ps = ps8.tile([128, 2, 512], f32, tag="sc", name="ps_s")
        nc.tensor.matmul(ps[:, 0, :], kT[0:64, hp, t2s], qT[0:64, hp, t1s],
                         start=True, stop=True)
        nc.tensor.matmul(ps[:, 1, :], kT[64:128, hp, t2s], qT[64:128, hp, t1s],
                         start=True, stop=True)
        y_t = yp.tile([128, 1024], bf16, tag="y")
        nc.scalar.activation(out=y_t, in_=ps.rearrange("p a b -> p (a b)"),
                             func=AF.Exp, scale=-1.0 / SCALE, bias=mu_val)
        g_t = gp.tile([128, 1024], bf16, tag="g")
        nc.vector._custom_dve(recip_act2, out=g_t, in0=y_t, s0=RA_C0, s1=RA_C1)
        g_tiles[step] = g_t
        # spread pending output-projection units into PE/ACT slack
        if bi >= 13 and t2t % 3 == 1 and c_queue:
            emit_c_unit(c_queue.pop(0))

    def emit_trail(step):
        bi, t2t = divmod(step, N_TT)
        hp, t1b = blocks[bi]
        g_t = g_tiles.pop(step)
        if t2t == 0:
            av_tiles[bi] = ps8.tile([128, 2, 512], f32, tag="av", name="av")
        av = av_tiles[bi]
        h_e, h_o = 2 * hp, 2 * hp + 1
        nc.tensor.matmul(
            av[0:65, 0, :], v_aug[:, t2t, h_e * VSTR:h_e * VSTR + 65],
            g_t[:, 0:512],
            start=(t2t == 0), stop=(t2t == N_TT - 1))
        nc.tensor.matmul(
            av[0:65, 1, :], v_aug[:, t2t, h_o * VSTR:h_o * VSTR + 65],
            g_t[:, 512:1024],
            start=(t2t == 0), stop=(t2t == N_TT - 1))
        if t2t == N_TT - 1:
            emit_norm(bi)
            if blocks[bi][0] == 3:
                c_queue.extend(range(4 * blocks[bi][1], 4 * blocks[bi][1] + 4))

    def emit_norm(bi):
        hp, t1b = blocks[bi]
        t1s = slice(t1b * 512, (t1b + 1) * 512)
        av = av_tiles.pop(bi)
        sb_av = nrm.tile([65, 2, 512], f32, tag="sb_av")
        nc.scalar.copy(out=sb_av, in_=av[0:65, :, :])
        den2 = nrm.tile([2, 512], f32, tag="den")
        nc.sync.dma_start(out=den2[0:1, :], in_=sb_av[64:65, 0, :])
        nc.sync.dma_start(out=den2[1:2, :], in_=sb_av[64:65, 1, :])
        r2 = nrm.tile([2, 512], bf16, tag="r")
        nc.vector._custom_dve(RECIPROCAL_APPROX_FAST, out=r2, in0=den2,
                              s0=rc["s0"], s1=rc["s1"], imm2=rc["imm2"])
        nc.sync.dma_start(out=rsc_d[bi], in_=r2)
        rbc = nrm.tile([64, 2, 512], bf16, tag="rbc")
        nc.sync.dma_start(out=rbc[:, 0, :],
                          in_=_broadcast_row(rsc_d[bi, 0:1, :], 64))
        nc.sync.dma_start(out=rbc[:, 1, :],
                          in_=_broadcast_row(rsc_d[bi, 1:2, :], 64))
        nc.gpsimd.tensor_tensor(out=attn_sb[0:64, hp, t1s],
                                in0=sb_av[0:64, 0, :], in1=rbc[:, 0, :],
                                op=ALU.mult)
        tmp = nrm.tile([64, 512], bf16, tag="tmp")
        nc.gpsimd.tensor_tensor(out=tmp, in0=sb_av[0:64, 1, :],
                                in1=rbc[:, 1, :], op=ALU.mult)
        nc.sync.dma_start(out=attn_sb[64:128, hp, t1s], in_=tmp)

    c_queue = []

    def emit_c_unit(tt2):
        """Output projection for one 128-token tile (both 512-wide halves)."""
        ps_o = ps8.tile([128, 2, 512], f32, tag="sc", name="ps_o")
        for et in range(2):
            es = slice(et * 512, (et + 1) * 512)
            for dt in range(4):
                nc.tensor.matmul(
                    ps_o[:, et, :],
                    attn_sb[:, dt, tt2 * 128:(tt2 + 1) * 128],
                    wo_sb[:, dt, es],
                    start=(dt == 0), stop=(dt == 3),
                )
        o_t = op_.tile([128, 1024], f32, tag="o")
        nc.scalar.copy(out=o_t, in_=ps_o.rearrange("p a b -> p (a b)"))
        nc.sync.dma_start(
            out=out_d[tt2 * 128:(tt2 + 1) * 128, :], in_=o_t)

    for step in range(n_steps + LAG):
        if step < n_steps:
            emit_lead(step)
        if step - LAG >= 0:
            emit_trail(step - LAG)
    while c_queue:
        emit_c_unit(c_queue.pop(0))


_PROGRAM_CACHE = {}


def _get_program(mu_val: float):
    key = round(float(mu_val), 9)
    if key not in _PROGRAM_CACHE:
        _PROGRAM_CACHE[key] = _build_program(float(mu_val))
    return _PROGRAM_CACHE[key]


def make_core_inputs(sequence, ln_gamma, ln_beta, Wq, bq, Wk, bk, Wv, bv, Wo, bo, mu):
    """Host-side shard prep: per-core input dicts (gamma/beta folded into W/b)."""
    f = np.float32
    bf = _ml.bfloat16
    seq = np.asarray(sequence, f)
    g = np.asarray(ln_gamma, f)
    be = np.asarray(ln_beta, f)
    in_maps = []
    for c in range(N_CORES):
        b, grp = c // 2, c % 2
        blk = slice(W_LOC * grp, W_LOC * (grp + 1))
        Wqb = np.asarray(Wq, f)[blk]
        Wkb = np.asarray(Wk, f)[blk]
        Wvb = np.asarray(Wv, f)[blk]
        m = {
            "x": np.ascontiguousarray(seq[b]),
            "wqT": np.ascontiguousarray((Wqb * g[None, :]).T).astype(bf),
            "wkT": np.ascontiguousarray((Wkb * g[None, :]).T).astype(bf),
            "wvT": np.ascontiguousarray((Wvb * g[None, :]).T).astype(bf),
            "woT": np.ascontiguousarray(np.asarray(Wo, f)[:, blk].T).astype(bf),
            "bq": np.ascontiguousarray(np.asarray(bq, f)[blk] + Wqb @ be),
            "bk": np.ascontiguousarray(np.asarray(bk, f)[blk] + Wkb @ be),
            "bv": np.ascontiguousarray(np.asarray(bv, f)[blk] + Wvb @ be),
        }
        in_maps.append(m)
    return in_maps


def combine_outputs(results, bo):
    out = np.zeros((BATCH, SEQ, D_MODEL), np.float32)
    for c in range(N_CORES):
        out[c // 2] += results[c]["out"]
    out += np.asarray(bo, np.float32)[None, None, :]
    return out


def kernel(sequence, ln_gamma, ln_beta, Wq, bq, Wk, bk, Wv, bv, Wo, bo, mu,
           _trace=False):
    from concourse.bass_utils import run_bass_kernel_spmd

    mu_val = float(np.asarray(mu).reshape(-1)[0])
    nc = _get_program(mu_val)
    in_maps = make_core_inputs(sequence, ln_gamma, ln_beta, Wq, bq, Wk, bk,
                               Wv, bv, Wo, bo, mu)
    res = run_bass_kernel_spmd(nc, in_maps, list(range(N_CORES)), trace=_trace)
    out = combine_outputs(res.results, bo)
    if _trace:
        kernel.last_results = res
    return out
